# revision 7
# baseline (speedup 1.0000x reference)
"""Bass/Trainium2 kernel for ExtractPatchesPosition (bilinear patch extraction).

Strategy (pure data parallel, batch sharded over 8 cores; 256 samples/core):

For each (sample b, channel c) the reference samples a translated N x N grid
out(r,col) = img(r + 32 + oy, col + 32 + ox) with bilinear interpolation.
With |offset| <= 20 and margin 32 the samples never leave the image, so the
whole patch is: take the (N+1) x (N+1) window at integer origin
(y0, x0) = (floor(32+oy), floor(32+ox)) and blend

    t = (1-fy)*W[r, x]   + fy*W[r+1, x]      (vertical 2-tap)
    o = (1-fx)*t[r, col] + fx*t[r, col+1]    (horizontal 2-tap)

Device pipeline, per group of 128 samples (partition = sample), one pass per
channel c (8 passes per core):
  1. one indirect DMA (SWDGE) gathers, per partition, a contiguous run of
     65*128 bf16 elements of the flat image starting at the window origin
     (s*128 + y0)*128 + x0.  Both data-dependent shifts are absorbed into the
     per-partition element-granularity start offset; inside the run the
     window sits at static offsets (r*128 + x).
  2. each 2-tap blend a*w + b*(1-w) is split as ACT mul (the odd-offset
     operand; ACT has no DVE fast mode to lose) + DVE tensor_scalar (4x
     packed-bf16 mode) + DVE tensor_tensor add (2x mode).  All DVE operands
     are packed with even inner dims; scalar_tensor_tensor is avoided since
     it supports no DVE fast modes.
  3. the output is stored channel-planar: per (group, channel) one HWDGE DMA
     writes o_c[128, 4096] -> out[s, c*4096:(c+1)*4096] (8 KiB contiguous
     per sample).  The host interleaves channels during the unshard
     (pure layout transform, no device work).

The whole datapath runs in bf16 (rel-err budget is 2e-2; bf16 contributes
~7e-3), halving both gather and store HBM traffic vs f32.  The tiny
per-window metadata (int window origins, fractional weights) is precomputed
on host from `positions` (O(B*C) work) and passed as extra input tensors;
all O(B*N*N*C) data movement and math runs on device.
"""

import numpy as np

import concourse.bacc as bacc
import concourse.tile as tile
from concourse import mybir
from concourse.bass import IndirectOffsetOnAxis

B, M, N, C = 2048, 128, 64, 4
NCORES = 8
BC = B // NCORES          # 256 samples per core
P = 128                   # samples per group (one per partition)
GROUPS = BC // P          # 2 groups per core
PASSES = GROUPS * C       # 8 channel-passes per core
ROWS = N + 1              # 65 window rows
RUN = ROWS * M            # 8320 gathered elements per window
TW = N + 2                # 66: vertical-blend width (even, for DVE 2x/4x)
OUTW = N * N * C          # 16384 out elements per sample
F32 = mybir.dt.float32
BF16 = mybir.dt.bfloat16
Copy = mybir.ActivationFunctionType.Copy
MULT = mybir.AluOpType.mult
ADD = mybir.AluOpType.add

_NC_CACHE = {}


def _build_nc():
    nc = bacc.Bacc("TRN2")
    img = nc.declare_dram_parameter("img", [BC * M * M, 1], BF16, isOutput=False)
    idx = nc.declare_dram_parameter("idx", [128, PASSES], mybir.dt.int32, isOutput=False)
    meta = nc.declare_dram_parameter("meta", [128, 4 * PASSES], F32, isOutput=False)
    out = nc.declare_dram_parameter("out", [BC, OUTW], BF16, isOutput=True)

    with tile.TileContext(nc) as tc:
        with (
            tc.tile_pool(name="singles", bufs=1) as singles,
            tc.tile_pool(name="gpool", bufs=3) as gpool,
            tc.tile_pool(name="tpool", bufs=2) as tpool,
            tc.tile_pool(name="uvpool", bufs=2) as uvpool,
            tc.tile_pool(name="abpool", bufs=2) as abpool,
            tc.tile_pool(name="opool", bufs=3) as opool,
        ):
            idx_sb = singles.tile([128, PASSES], mybir.dt.int32)
            meta_sb = singles.tile([128, 4 * PASSES], F32)
            nc.sync.dma_start(idx_sb[:], idx[:])
            nc.sync.dma_start(meta_sb[:], meta[:])

            for g in range(GROUPS):
                for c in range(C):
                    ps = g * C + c
                    # -- 1. gather: one contiguous RUN per window -----------
                    G = gpool.tile([128, RUN], BF16, tag="G")
                    nc.gpsimd.indirect_dma_start(
                        out=G[:],
                        out_offset=None,
                        in_=img[:],
                        in_offset=IndirectOffsetOnAxis(
                            ap=idx_sb[:, ps : ps + 1], axis=0
                        ),
                    )
                    Gv = G[:].rearrange("p (r x) -> p r x", x=M)  # [128, 65, 128]

                    fy1 = meta_sb[:, 4 * ps + 0 : 4 * ps + 1]  # 1 - fy
                    fy = meta_sb[:, 4 * ps + 1 : 4 * ps + 2]
                    fx1 = meta_sb[:, 4 * ps + 2 : 4 * ps + 3]  # 1 - fx
                    fx = meta_sb[:, 4 * ps + 3 : 4 * ps + 4]

                    # -- 2. vertical blend: t = (1-fy)*W[r] + fy*W[r+1],
                    #    on TW=66 columns (even inner dim) ------------------
                    u = uvpool.tile([128, N * TW], BF16, tag="uv")
                    uv = u[:].rearrange("p (r x) -> p r x", x=TW)
                    nc.scalar.activation(uv, Gv[:, 1:ROWS, 0:TW], Copy, scale=fy)
                    a = abpool.tile([128, N * TW], BF16, tag="ab")
                    av = a[:].rearrange("p (r x) -> p r x", x=TW)
                    nc.vector.tensor_scalar_mul(av, Gv[:, 0:N, 0:TW], fy1)
                    t = tpool.tile([128, N * TW], BF16, tag="t")
                    nc.vector.tensor_tensor(t[:], a[:], u[:], ADD)
                    tv = t[:].rearrange("p (r x) -> p r x", x=TW)

                    # -- horizontal blend: o = (1-fx)*t[col] + fx*t[col+1] --
                    v = uvpool.tile([128, N * N], BF16, tag="uv")
                    vv = v[:].rearrange("p (r x) -> p r x", x=N)
                    nc.scalar.activation(vv, tv[:, :, 1 : N + 1], Copy, scale=fx)
                    b = abpool.tile([128, N * N], BF16, tag="ab")
                    bv = b[:].rearrange("p (r x) -> p r x", x=N)
                    nc.vector.tensor_scalar_mul(bv, tv[:, :, 0:N], fx1)
                    o = opool.tile([128, N * N], BF16, tag="o")
                    nc.vector.tensor_tensor(o[:], b[:], v[:], ADD)

                    # -- 3. store: 8 KiB contiguous per sample --------------
                    nc.sync.dma_start(
                        out=out[g * P : (g + 1) * P, c * N * N : (c + 1) * N * N],
                        in_=o[:],
                    )
    nc.finalize()
    return nc


def get_nc():
    if "nc" not in _NC_CACHE:
        _NC_CACHE["nc"] = _build_nc()
    return _NC_CACHE["nc"]


def make_core_inputs(padded_obj, positions):
    """Host-side prep: shard + window metadata. Returns list of in_maps."""
    import ml_dtypes

    padded_obj = np.asarray(padded_obj, dtype=np.float32)
    positions = np.asarray(positions, dtype=np.float32)
    ox = positions[:, 0, 0, :]  # [B, C] column offsets
    oy = positions[:, 0, 1, :]  # [B, C] row offsets
    c0 = np.float32((M - N) // 2)
    sx = (c0 + ox).astype(np.float32)
    sy = (c0 + oy).astype(np.float32)
    x0 = np.floor(sx).astype(np.int32)
    y0 = np.floor(sy).astype(np.int32)
    fx = (sx - x0.astype(np.float32)).astype(np.float32)
    fy = (sy - y0.astype(np.float32)).astype(np.float32)

    img_bf = padded_obj[:, :, :, 0].astype(ml_dtypes.bfloat16)

    in_maps = []
    for core in range(NCORES):
        s = slice(core * BC, (core + 1) * BC)
        img_c = np.ascontiguousarray(img_bf[s]).reshape(-1, 1)
        y0c, x0c = y0[s], x0[s]
        fyc, fxc = fy[s], fx[s]
        idx_c = np.empty((128, PASSES), np.int32)
        meta_c = np.empty((128, 4 * PASSES), np.float32)
        p = np.arange(128)
        for g in range(GROUPS):
            sloc = g * P + p
            for c in range(C):
                ps = g * C + c
                idx_c[:, ps] = (sloc * M + y0c[sloc, c]) * M + x0c[sloc, c]
                meta_c[:, 4 * ps + 0] = np.float32(1.0) - fyc[sloc, c]
                meta_c[:, 4 * ps + 1] = fyc[sloc, c]
                meta_c[:, 4 * ps + 2] = np.float32(1.0) - fxc[sloc, c]
                meta_c[:, 4 * ps + 3] = fxc[sloc, c]
        in_maps.append({"img": img_c, "idx": idx_c, "meta": meta_c})
    return in_maps


def _make_runner(nc):
    """Build a persistent jitted SPMD executor for `nc` (compiles once).

    Mirrors concourse.bass2jax.run_bass_via_pjrt but caches the jitted
    function so repeated kernel() calls don't re-trigger neuronx-cc.
    """
    import jax
    from jax.sharding import Mesh, PartitionSpec
    from jax.experimental.shard_map import shard_map
    from concourse import bass2jax, mybir as mb

    bass2jax.install_neuronx_cc_hook()
    assert not nc.dbg_callbacks, "dbg callbacks unsupported under axon"

    extra_in_maps = {}
    if nc.dbg_addr is not None:
        extra_in_maps[nc.dbg_addr.name] = np.zeros((1, 2), np.uint32)
    partition_name = nc.partition_id_tensor.name if nc.partition_id_tensor else None

    in_names, out_names, out_avals = [], [], []
    for alloc in nc.m.functions[0].allocations:
        if not isinstance(alloc, mb.MemoryLocationSet):
            continue
        name = alloc.memorylocations[0].name
        if alloc.kind == "ExternalInput":
            if name != partition_name:
                in_names.append(name)
        elif alloc.kind == "ExternalOutput":
            out_names.append(name)
            out_avals.append(
                jax.core.ShapedArray(tuple(alloc.tensor_shape), mb.dt.np(alloc.dtype))
            )
    n_params = len(in_names)
    n_outs = len(out_avals)
    all_names = in_names + out_names
    if partition_name is not None:
        all_names = all_names + [partition_name]
    donate = tuple(range(n_params, n_params + n_outs))

    def _body(*args):
        operands = list(args)
        if partition_name is not None:
            operands.append(bass2jax.partition_id_tensor())
        outs = bass2jax._bass_exec_p.bind(
            *operands,
            out_avals=tuple(out_avals),
            in_names=tuple(all_names),
            out_names=tuple(out_names),
            lowering_input_output_aliases=(),
            sim_require_finite=True,
            sim_require_nnan=True,
            nc=nc,
        )
        return tuple(outs)

    devices = jax.devices()[:NCORES]
    mesh = Mesh(np.asarray(devices), ("core",))
    in_specs = (PartitionSpec("core"),) * (n_params + n_outs)
    out_specs = (PartitionSpec("core"),) * n_outs
    sharded = jax.jit(
        shard_map(_body, mesh=mesh, in_specs=in_specs, out_specs=out_specs,
                  check_rep=False),
        donate_argnums=donate,
        keep_unused=True,
    )

    def run(in_maps, device_only=False):
        if extra_in_maps:
            in_maps = [{**m, **extra_in_maps} for m in in_maps]
        concat_in = [
            np.concatenate([np.asarray(m[name]) for m in in_maps], axis=0)
            for name in in_names
        ]
        concat_zeros = [
            np.zeros((NCORES * a.shape[0], *a.shape[1:]), a.dtype) for a in out_avals
        ]
        out_arrs = sharded(*concat_in, *concat_zeros)
        if device_only:
            jax.block_until_ready(out_arrs)
            return None
        return {
            name: np.asarray(out_arrs[i]) for i, name in enumerate(out_names)
        }

    return run


def get_runner():
    if "run" not in _NC_CACHE:
        _NC_CACHE["run"] = _make_runner(get_nc())
    return _NC_CACHE["run"]


def kernel(padded_obj, positions, N=None):
    assert padded_obj.shape == (B, M, M, 1), padded_obj.shape
    in_maps = make_core_inputs(padded_obj, positions)
    out = get_runner()(in_maps)["out"]
    # device layout is channel-planar [b, c, r, col] -> NHWC
    return np.ascontiguousarray(
        out.astype(np.float32).reshape(B, C, 64, 64).transpose(0, 2, 3, 1)
    )


# revision 9
# speedup vs baseline: 1.4874x; 1.4874x over previous
"""Bass/Trainium2 kernel for ExtractPatchesPosition (bilinear patch extraction).

Strategy (pure data parallel, batch sharded over 8 cores; 256 samples/core):

For each (sample b, channel c) the reference samples a translated N x N grid
out(r,col) = img(r + 32 + oy, col + 32 + ox) with bilinear interpolation.
With |offset| <= 20 and margin 32 the samples never leave the image, so the
whole patch is: take the (N+1) x (N+1) window at integer origin
(y0, x0) = (floor(32+oy), floor(32+ox)) and blend

    t = (1-fy)*W[r, x]   + fy*W[r+1, x]      (vertical 2-tap)
    o = (1-fx)*t[r, col] + fx*t[r, col+1]    (horizontal 2-tap)

Device pipeline, per group of 128 samples (partition = sample), one pass per
channel c (8 passes per core):
  1. one indirect DMA (SWDGE) gathers, per partition, a contiguous run of
     65*128 bf16 elements of the flat image starting at the window origin
     (s*128 + y0)*128 + x0.  Both data-dependent shifts are absorbed into the
     per-partition element-granularity start offset; inside the run the
     window sits at static offsets (r*128 + x).
  2. each 2-tap blend a*w + b*(1-w) is split as ACT mul (the odd-offset
     operand; ACT has no DVE fast mode to lose) + DVE tensor_scalar (4x
     packed-bf16 mode) + DVE tensor_tensor add (2x mode).  All DVE operands
     are packed with even inner dims; scalar_tensor_tensor is avoided since
     it supports no DVE fast modes.
  3. the output is stored channel-planar: per (group, channel) one HWDGE DMA
     writes o_c[128, 4096] -> out[s, c*4096:(c+1)*4096] (8 KiB contiguous
     per sample).  The host interleaves channels during the unshard
     (pure layout transform, no device work).

The whole datapath runs in bf16 (rel-err budget is 2e-2; bf16 contributes
~7e-3), halving both gather and store HBM traffic vs f32.  The tiny
per-window metadata (int window origins, fractional weights) is precomputed
on host from `positions` (O(B*C) work) and passed as extra input tensors;
all O(B*N*N*C) data movement and math runs on device.
"""

import numpy as np

import concourse.bacc as bacc
import concourse.tile as tile
from concourse import mybir
from concourse.bass import IndirectOffsetOnAxis

B, M, N, C = 2048, 128, 64, 4
NCORES = 8
BC = B // NCORES          # 256 samples per core
P = 128                   # samples per group (one per partition)
GROUPS = BC // P          # 2 groups per core
PASSES = GROUPS * C       # 8 channel-passes per core
ROWS = N + 1              # 65 window rows
RUN = ROWS * M            # 8320 gathered elements per window
TW = N + 2                # 66: vertical-blend width (even, for DVE 2x/4x)
OUTW = N * N * C          # 16384 out elements per sample
F32 = mybir.dt.float32
BF16 = mybir.dt.bfloat16
Copy = mybir.ActivationFunctionType.Copy
MULT = mybir.AluOpType.mult
ADD = mybir.AluOpType.add

_NC_CACHE = {}


def _build_nc():
    nc = bacc.Bacc("TRN2")
    img = nc.declare_dram_parameter("img", [BC * M * M, 1], BF16, isOutput=False)
    idx = nc.declare_dram_parameter("idx", [128, PASSES], mybir.dt.int32, isOutput=False)
    meta = nc.declare_dram_parameter("meta", [128, 4 * PASSES], F32, isOutput=False)
    out = nc.declare_dram_parameter("out", [BC, OUTW], BF16, isOutput=True)

    with tile.TileContext(nc) as tc:
        with (
            tc.tile_pool(name="singles", bufs=1) as singles,
            tc.tile_pool(name="gpool", bufs=3) as gpool,
            tc.tile_pool(name="tpool", bufs=2) as tpool,
            tc.tile_pool(name="uvpool", bufs=4) as uvpool,
            tc.tile_pool(name="abpool", bufs=2) as abpool,
            tc.tile_pool(name="opool", bufs=3) as opool,
        ):
            idx_sb = singles.tile([128, PASSES], mybir.dt.int32)
            meta_sb = singles.tile([128, 4 * PASSES], F32)
            nc.sync.dma_start(idx_sb[:], idx[:])
            nc.sync.dma_start(meta_sb[:], meta[:])

            HB = N // 2      # 32 rows per half-pass
            HBE = HB * M     # gather elements per 32 rows

            def gather(ps, Gdst, element_offset=0):
                nc.gpsimd.indirect_dma_start(
                    out=Gdst,
                    out_offset=None,
                    in_=img[:],
                    in_offset=IndirectOffsetOnAxis(ap=idx_sb[:, ps : ps + 1], axis=0),
                    element_offset=element_offset,
                )

            def scales(ps):
                return (
                    meta_sb[:, 4 * ps + 0 : 4 * ps + 1],  # 1 - fy
                    meta_sb[:, 4 * ps + 1 : 4 * ps + 2],  # fy
                    meta_sb[:, 4 * ps + 2 : 4 * ps + 3],  # 1 - fx
                    meta_sb[:, 4 * ps + 3 : 4 * ps + 4],  # fx
                )

            # state carried between pipeline stages, keyed by pass
            tiles = {}

            def emit_u(ps, r0, r1, Gt):
                # u[r] = fy * W[r+1], rows r0:r1 (ACT; odd row offset is fine)
                fy = scales(ps)[1]
                u = uvpool.tile([128, (r1 - r0) * TW], BF16, tag="uv")
                uv = u[:].rearrange("p (r x) -> p r x", x=TW)
                Gvw = Gt[:].rearrange("p (r x) -> p r x", x=M)
                nc.scalar.activation(
                    uv, Gvw[:, r0 + 1 : r1 + 1, 0:TW], Copy, scale=fy
                )
                return u

            def emit_vert(ps, r0, r1, Gt, u, t):
                # t[r0:r1] = (1-fy)*W[r] + u   (DVE ts 4x + tt 2x)
                fy1 = scales(ps)[0]
                nr = r1 - r0
                Gvw = Gt[:].rearrange("p (r x) -> p r x", x=M)
                a = abpool.tile([128, nr * TW], BF16, tag="ab")
                av = a[:].rearrange("p (r x) -> p r x", x=TW)
                nc.vector.tensor_scalar_mul(av, Gvw[:, r0:r1, 0:TW], fy1)
                nc.vector.tensor_tensor(
                    t[:, r0 * TW : r1 * TW], a[:], u[:], ADD
                )

            def emit_horiz(ps, r0, r1, t, o):
                # o[r0:r1] = (1-fx)*t[col] + fx*t[col+1]
                fy1, fy, fx1, fx = scales(ps)
                nr = r1 - r0
                tv = t[:, r0 * TW : r1 * TW].rearrange("p (r x) -> p r x", x=TW)
                v = uvpool.tile([128, nr * N], BF16, tag="uv")
                vv = v[:].rearrange("p (r x) -> p r x", x=N)
                nc.scalar.activation(vv, tv[:, :, 1 : N + 1], Copy, scale=fx)
                b = abpool.tile([128, nr * N], BF16, tag="ab")
                bv = b[:].rearrange("p (r x) -> p r x", x=N)
                nc.vector.tensor_scalar_mul(bv, tv[:, :, 0:N], fx1)
                nc.vector.tensor_tensor(
                    o[:, r0 * N : r1 * N], b[:], v[:], ADD
                )

            def emit_store(ps, o, r0=0, r1=N):
                g, c = divmod(ps, C)
                nc.sync.dma_start(
                    out=out[
                        g * P : (g + 1) * P,
                        c * N * N + r0 * N : c * N * N + r1 * N,
                    ],
                    in_=o[:, r0 * N : r1 * N],
                )

            # ---- prologue: pass 0 gathered in two halves ------------------
            G0 = gpool.tile([128, RUN], BF16, tag="G")
            gather(0, G0[:, 0 : (HB + 1) * M])                    # rows 0:33
            gather(0, G0[:, (HB + 1) * M : RUN], (HB + 1) * M)    # rows 33:65
            G1 = gpool.tile([128, RUN], BF16, tag="G")
            gather(1, G1[:])
            tiles[0] = G0
            tiles[1] = G1
            t0 = tpool.tile([128, N * TW], BF16, tag="t")
            o0 = opool.tile([128, N * N], BF16, tag="o")

            # pass 0, half 1 (rows 0:32) — fills the pipeline fast
            u0a = emit_u(0, 0, HB, G0)
            emit_vert(0, 0, HB, G0, u0a, t0)
            u0b = emit_u(0, HB, N, G0)
            emit_horiz(0, 0, HB, t0, o0)
            emit_vert(0, HB, N, G0, u0b, t0)
            u_next = emit_u(1, 0, N, G1)
            emit_horiz(0, HB, N, t0, o0)
            emit_store(0, o0)

            # ---- steady passes 1..6, software-pipelined -------------------
            for ps in range(1, PASSES - 1):
                Gn = gpool.tile([128, RUN], BF16, tag="G")
                gather(ps + 1, Gn[:])
                tiles[ps + 1] = Gn
                t = tpool.tile([128, N * TW], BF16, tag="t")
                o = opool.tile([128, N * N], BF16, tag="o")
                emit_vert(ps, 0, N, tiles[ps], u_next, t)
                u_next = emit_u(ps + 1, 0, N, Gn)
                emit_horiz(ps, 0, N, t, o)
                emit_store(ps, o)

            # ---- final pass 7, split for a short tail ---------------------
            ps = PASSES - 1
            Gl = tiles[ps]
            t = tpool.tile([128, N * TW], BF16, tag="t")
            o = opool.tile([128, N * N], BF16, tag="o")
            emit_vert(ps, 0, N, Gl, u_next, t)
            emit_horiz(ps, 0, HB, t, o)
            emit_store(ps, o, 0, HB)
            emit_horiz(ps, HB, N, t, o)
            emit_store(ps, o, HB, N)
    nc.finalize()
    return nc


def get_nc():
    if "nc" not in _NC_CACHE:
        _NC_CACHE["nc"] = _build_nc()
    return _NC_CACHE["nc"]


def make_core_inputs(padded_obj, positions):
    """Host-side prep: shard + window metadata. Returns list of in_maps."""
    import ml_dtypes

    padded_obj = np.asarray(padded_obj, dtype=np.float32)
    positions = np.asarray(positions, dtype=np.float32)
    ox = positions[:, 0, 0, :]  # [B, C] column offsets
    oy = positions[:, 0, 1, :]  # [B, C] row offsets
    c0 = np.float32((M - N) // 2)
    sx = (c0 + ox).astype(np.float32)
    sy = (c0 + oy).astype(np.float32)
    x0 = np.floor(sx).astype(np.int32)
    y0 = np.floor(sy).astype(np.int32)
    fx = (sx - x0.astype(np.float32)).astype(np.float32)
    fy = (sy - y0.astype(np.float32)).astype(np.float32)

    img_bf = padded_obj[:, :, :, 0].astype(ml_dtypes.bfloat16)

    in_maps = []
    for core in range(NCORES):
        s = slice(core * BC, (core + 1) * BC)
        img_c = np.ascontiguousarray(img_bf[s]).reshape(-1, 1)
        y0c, x0c = y0[s], x0[s]
        fyc, fxc = fy[s], fx[s]
        idx_c = np.empty((128, PASSES), np.int32)
        meta_c = np.empty((128, 4 * PASSES), np.float32)
        p = np.arange(128)
        for g in range(GROUPS):
            sloc = g * P + p
            for c in range(C):
                ps = g * C + c
                idx_c[:, ps] = (sloc * M + y0c[sloc, c]) * M + x0c[sloc, c]
                meta_c[:, 4 * ps + 0] = np.float32(1.0) - fyc[sloc, c]
                meta_c[:, 4 * ps + 1] = fyc[sloc, c]
                meta_c[:, 4 * ps + 2] = np.float32(1.0) - fxc[sloc, c]
                meta_c[:, 4 * ps + 3] = fxc[sloc, c]
        in_maps.append({"img": img_c, "idx": idx_c, "meta": meta_c})
    return in_maps


def _make_runner(nc):
    """Build a persistent jitted SPMD executor for `nc` (compiles once).

    Mirrors concourse.bass2jax.run_bass_via_pjrt but caches the jitted
    function so repeated kernel() calls don't re-trigger neuronx-cc.
    """
    import jax
    from jax.sharding import Mesh, PartitionSpec
    from jax.experimental.shard_map import shard_map
    from concourse import bass2jax, mybir as mb

    bass2jax.install_neuronx_cc_hook()
    assert not nc.dbg_callbacks, "dbg callbacks unsupported under axon"

    extra_in_maps = {}
    if nc.dbg_addr is not None:
        extra_in_maps[nc.dbg_addr.name] = np.zeros((1, 2), np.uint32)
    partition_name = nc.partition_id_tensor.name if nc.partition_id_tensor else None

    in_names, out_names, out_avals = [], [], []
    for alloc in nc.m.functions[0].allocations:
        if not isinstance(alloc, mb.MemoryLocationSet):
            continue
        name = alloc.memorylocations[0].name
        if alloc.kind == "ExternalInput":
            if name != partition_name:
                in_names.append(name)
        elif alloc.kind == "ExternalOutput":
            out_names.append(name)
            out_avals.append(
                jax.core.ShapedArray(tuple(alloc.tensor_shape), mb.dt.np(alloc.dtype))
            )
    n_params = len(in_names)
    n_outs = len(out_avals)
    all_names = in_names + out_names
    if partition_name is not None:
        all_names = all_names + [partition_name]
    donate = tuple(range(n_params, n_params + n_outs))

    def _body(*args):
        operands = list(args)
        if partition_name is not None:
            operands.append(bass2jax.partition_id_tensor())
        outs = bass2jax._bass_exec_p.bind(
            *operands,
            out_avals=tuple(out_avals),
            in_names=tuple(all_names),
            out_names=tuple(out_names),
            lowering_input_output_aliases=(),
            sim_require_finite=True,
            sim_require_nnan=True,
            nc=nc,
        )
        return tuple(outs)

    devices = jax.devices()[:NCORES]
    mesh = Mesh(np.asarray(devices), ("core",))
    in_specs = (PartitionSpec("core"),) * (n_params + n_outs)
    out_specs = (PartitionSpec("core"),) * n_outs
    sharded = jax.jit(
        shard_map(_body, mesh=mesh, in_specs=in_specs, out_specs=out_specs,
                  check_rep=False),
        donate_argnums=donate,
        keep_unused=True,
    )

    def run(in_maps, device_only=False):
        if extra_in_maps:
            in_maps = [{**m, **extra_in_maps} for m in in_maps]
        concat_in = [
            np.concatenate([np.asarray(m[name]) for m in in_maps], axis=0)
            for name in in_names
        ]
        concat_zeros = [
            np.zeros((NCORES * a.shape[0], *a.shape[1:]), a.dtype) for a in out_avals
        ]
        out_arrs = sharded(*concat_in, *concat_zeros)
        if device_only:
            jax.block_until_ready(out_arrs)
            return None
        return {
            name: np.asarray(out_arrs[i]) for i, name in enumerate(out_names)
        }

    return run


def get_runner():
    if "run" not in _NC_CACHE:
        _NC_CACHE["run"] = _make_runner(get_nc())
    return _NC_CACHE["run"]


def kernel(padded_obj, positions, N=None):
    assert padded_obj.shape == (B, M, M, 1), padded_obj.shape
    in_maps = make_core_inputs(padded_obj, positions)
    out = get_runner()(in_maps)["out"]
    # device layout is channel-planar [b, c, r, col] -> NHWC
    return np.ascontiguousarray(
        out.astype(np.float32).reshape(B, C, 64, 64).transpose(0, 2, 3, 1)
    )
